# revision 1
# baseline (speedup 1.0000x reference)
"""nn_MultiHeadAttention_84954453115654 — Trainium2 Bass kernel, 8 NeuronCores.

Sharding: data-parallel over batch (2) x head-pair-parallel (4 groups of 2
heads).  Core c handles batch b = c//4 and embed-channel rows
R = [128*(c%4), 128*(c%4)+128) (= heads 2*(c%4) and 2*(c%4)+1).

Per core:
  - weight-standardize Wq/Wk/Wv row-block [128,512] and full Wo (stats over
    full rows, then slice the R columns), PE transposes for matmul layouts
  - projections (fp32r matmuls, K-chained over 4 tiles of 128); the column
    mask (x * mask) commutes through the 1x1 conv and is applied at the
    PSUM->SBUF drain (mask * psum); conv bias added after (matches ref)
  - per-head LayerNorm over DH=64 using a block-mean matmul broadcast:
    mu_b = M2^T @ x and ex2_b = M2^T @ x^2 where M2 is the per-head 1/64
    block matrix, then var/rsqrt/apply elementwise
  - scores computed TRANSPOSED: S'^T[tk,tq] = sum_d kn[d,tk] qn[d,tq] with
    1/SCALE folded into qn; key mask folded into the softmax exp as a
    per-partition bias (-80 on masked keys; exp(-80)~1e-35 which matches
    the reference's exact zeros within fp32); no row-max subtraction
    (post-LN scores are O(0.1) so exp never overflows)
  - av = V_aug^T @ exp(S') accumulated over tk tiles in PSUM; V_aug carries
    a ones column so row 64 of the result is the softmax denominator;
    the query mask is folded into the reciprocal row (this is where the
    reference's scores-row zeroing + out_proj input masking both land)
  - out_proj partial: wnoT[:, R-cols] @ (av / denom) -> [512, 2048]
Host: sums the 4 partials of each batch group and adds bo (the row-parallel
reduction of the sharding hint, performed at gather/unshard time).
"""

import os
import sys
import contextlib
import functools

for _p in ("/root/.axon_site/_ro/trn_rl_repo", "/opt/trn_rl_repo"):
    if os.path.isdir(_p) and _p not in sys.path:
        sys.path.append(_p)

import numpy as np

import concourse.bass as bass
import concourse.mybir as mybir
import concourse.tile as tile
from concourse import bass_utils, library_config

B, E, T, H = 2, 512, 2048, 8
DH = E // H            # 64
HPC = 2                # heads per core
G = 4                  # cores per batch group
NCORES = 8
NK = E // 128          # 4 contraction tiles
NJ = T // 512          # 4 time chunks
NI = T // 128          # 16 tk tiles
EPS = 1e-5
SCALE = float(E // H ** 0.5)   # 181.0
BIG = 80.0
FP = mybir.dt.float32
FR = mybir.dt.float32r
AF = mybir.ActivationFunctionType
OP = mybir.AluOpType


def _fr(ap):
    return ap.bitcast(FR)


def _split_multiwaits(nc):
    """Split multi-wait instructions (Tile's tail drain) into single-wait
    EventSemaphore chains; this container's walrus encodes only one sync
    wait per instruction."""
    import bass_rust

    n_new = 0
    for f in nc.m.functions:
        for bb in f.blocks:
            out = []
            changed = False
            for ins in bb.instructions:
                si = ins.sync_info
                if si is not None and si.on_wait is not None and len(si.on_wait) > 1:
                    waits = list(si.on_wait)
                    for w in waits[:-1]:
                        ev = bass_rust.InstEventSemaphore(
                            name=f"MWFIX-{n_new}", ins=[], outs=[]
                        )
                        n_new += 1
                        ev.engine = ins.engine
                        ev.sync_info = bass_rust.SyncInfo(on_wait=[w], on_update=[])
                        out.append(ev)
                    ins.sync_info = bass_rust.SyncInfo(
                        on_wait=[waits[-1]], on_update=list(si.on_update or [])
                    )
                    changed = True
                out.append(ins)
            if changed:
                bb.instructions = out
    return n_new


def _emit(nc, tc, dram, flags, dbg, reps=1):
    v = nc.vector
    sc = nc.scalar
    te = nc.tensor
    gp = nc.gpsimd
    sy = nc.sync

    stack = contextlib.ExitStack()
    pools = {}

    def pool(name, bufs, space="SBUF"):
        if name not in pools:
            pools[name] = stack.enter_context(
                tc.tile_pool(name=name, bufs=bufs, space=space)
            )
        return pools[name]

    consts = pool("consts", 1)
    wbuf = pool("wbuf", 1)
    mbc = pool("maskb", 1)
    xp = pool("x", 20)
    tpp = pool("tp", 1)
    sqp = pool("sq", 2)
    scr = pool("scr", 2)
    stat = pool("stat", 2)
    nbuf = pool("named", 1)
    nump = pool("numer", 4)
    rsbp = pool("rsb", 2)
    rbp = pool("rb", 6)
    outp = pool("outsb", 4)
    dramp = pool("dram_scr", 2, "DRAM")

    # ---- constant / weight loads -------------------------------------
    ident = consts.tile([128, 128], FP, tag="ident")
    sy.dma_start(ident[:], dram["ident"])
    identr = consts.tile([128, 128], FR, tag="identr")
    sy.dma_start(identr[:], dram["identr"])
    m2t = consts.tile([128, 128], FR, tag="m2")
    sy.dma_start(m2t[:], dram["m2"])
    kmt = consts.tile([128, NI], FP, tag="kmt")
    sy.dma_start(kmt[:], dram["kmt"])

    # query mask row persists (used again at softmax normalization)
    qmrow = consts.tile([1, T], FP, tag="qmrow")
    sy.dma_start(qmrow[:], dram["qm"])

    wtiles = {}
    for wname in ("wq", "wk", "wv"):
        wt = consts.tile([128, E], FP, tag=wname)
        sy.dma_start(wt[:], dram[wname])
        wtiles[wname] = wt
    wo_tiles = []
    for m in range(4):
        wt = consts.tile([128, E], FP, tag=f"wo{m}")
        sy.dma_start(wt[:], dram["wo"][128 * m : 128 * (m + 1), :])
        wo_tiles.append(wt)
    # NOTE: W tiles stay in consts (7 x 2KB); acceptable.

    bias_tiles = {}
    if flags["use_bias"]:
        for bname in ("bq", "bk", "bv"):
            bt = consts.tile([128, 1], FP, tag=bname)
            sy.dma_start(bt[:], dram[bname])
            bias_tiles[bname] = bt
    ge_tiles = {}
    if flags["use_affine"]:
        for gname in ("geq", "beq", "gek", "bek", "gev", "bev"):
            gt = consts.tile([128, 1], FP, tag=gname)
            sy.dma_start(gt[:], dram[gname])
            ge_tiles[gname] = gt

    # key-mask exp bias: (km-1)*BIG  [128, NI]
    mbias = consts.tile([128, NI], FP, tag="mbias")
    v.tensor_scalar(mbias[:], kmt[:], 1.0, BIG, op0=OP.subtract, op1=OP.mult)

    epst = consts.tile([128, 1], FP, tag="eps")
    v.memset(epst[:], EPS)

    ones64 = consts.tile([1, 64], FR, tag="ones64")
    sy.dma_start(ones64[:], dram["ones64"])

    pools.update(
        t_ident=ident, t_identr=identr, t_m2=m2t, t_kmt=kmt, t_qmrow=qmrow,
        t_wtiles=wtiles, t_wo_tiles=wo_tiles, t_bias_tiles=bias_tiles,
        t_ge_tiles=ge_tiles, t_mbias=mbias, t_eps=epst, t_ones64=ones64,
    )

    def emit_body():
        _emit_body(nc, tc, dram, flags, dbg, pools)

    for _rep in range(reps):
        emit_body()

    stack.close()


def _emit_body(nc, tc, dram, flags, dbg, pools):
    v = nc.vector
    sc = nc.scalar
    te = nc.tensor
    gp = nc.gpsimd
    sy = nc.sync
    consts = pools["consts"]
    wbuf = pools["wbuf"]
    mbc = pools["maskb"]
    xp = pools["x"]
    tpp = pools["tp"]
    sqp = pools["sq"]
    scr = pools["scr"]
    stat = pools["stat"]
    nbuf = pools["named"]
    nump = pools["numer"]
    rsbp = pools["rsb"]
    rbp = pools["rb"]
    outp = pools["outsb"]
    dramp = pools["dram_scr"]
    ident = pools["t_ident"]
    identr = pools["t_identr"]
    m2t = pools["t_m2"]
    kmt = pools["t_kmt"]
    qmrow = pools["t_qmrow"]
    wtiles = pools["t_wtiles"]
    wo_tiles = pools["t_wo_tiles"]
    bias_tiles = pools["t_bias_tiles"]
    ge_tiles = pools["t_ge_tiles"]
    mbias = pools["t_mbias"]
    epst = pools["t_eps"]
    ones64 = pools["t_ones64"]

    # ---- weight standardization + transposes -------------------------
    def w_standardize(wt, col_lo, col_n):
        s1 = stat.tile([128, 1], FP, tag="ws1")
        v.reduce_sum(s1[:], wt[:], axis=mybir.AxisListType.X)
        wsq = sqp.tile([128, E], FP, tag="wn")
        sc.activation(wsq[:], wt[:], AF.Square)
        s2 = stat.tile([128, 1], FP, tag="ws2")
        v.reduce_sum(s2[:], wsq[:], axis=mybir.AxisListType.X)
        mu = stat.tile([128, 1], FP, tag="wmu")
        gp.tensor_scalar_mul(mu[:], s1[:], 1.0 / E)
        ex2 = stat.tile([128, 1], FP, tag="wex2")
        gp.tensor_scalar_mul(ex2[:], s2[:], 1.0 / E)
        msq = stat.tile([128, 1], FP, tag="wmsq")
        gp.tensor_mul(msq[:], mu[:], mu[:])
        var = stat.tile([128, 1], FP, tag="wvar")
        gp.tensor_sub(var[:], ex2[:], msq[:])
        sd = stat.tile([128, 1], FP, tag="wsd")
        sc.activation(sd[:], var[:], AF.Sqrt, bias=epst[:])
        rsq = stat.tile([128, 1], FP, tag="wrsq")
        v.reciprocal(rsq[:], sd[:])
        wn = sqp.tile([128, col_n], FR, tag="wn")
        gp.tensor_scalar(
            wn[:],
            wt[:, col_lo : col_lo + col_n],
            mu[:],
            rsq[:],
            op0=OP.subtract,
            op1=OP.mult,
        )
        return wn

    # For q/k/v fold the per-head centering (I - M2) into the weights so the
    # projection matmul directly produces diff = p - mean_head(p):
    # (I - M2) @ (Wn @ x) = ((I - M2) Wn) @ x, and the t-column mask commutes.
    wT = {}
    with tc.tile_pool(name="ptrw", bufs=2, space="PSUM") as ptrw:
        for wname in ("wq", "wk", "wv"):
            wn = w_standardize(wtiles[wname], 0, E)
            if not flags["use_bias"]:
                pwc = ptrw.tile([128, E], FP, tag="wc")
                te.matmul(pwc[:], m2t[:], wn[:], start=True, stop=True)
                wc = sqp.tile([128, E], FR, tag="wn")
                v.tensor_sub(wc[:], wn[:], pwc[:])
            else:
                wc = wn
            wTt = wbuf.tile([128, E], FR, tag=f"{wname}T")
            for k in range(NK):
                pt = ptrw.tile([128, 128], FR, tag="wtr")
                te.transpose(pt[:], wc[:, 128 * k : 128 * (k + 1)], identr[:])
                sc.copy(wTt[:, 128 * k : 128 * (k + 1)], pt[:])
            wT[wname] = wTt

        # wo columns for this core's R were moved to the front on the host,
        # so the device always slices cols [0:128).
        woT = wbuf.tile([128, E], FR, tag="woT")
        for m in range(4):
            wn = w_standardize(wo_tiles[m], 0, 128)
            pt = ptrw.tile([128, 128], FR, tag="wtr")
            te.transpose(pt[:], wn[:], identr[:])
            sc.copy(woT[:, 128 * m : 128 * (m + 1)], pt[:])

    # value-mask broadcast early so it doesn't queue behind the x loads
    vm_mbt = mbc.tile([128, T], FP, tag="mb")
    sy.dma_start(vm_mbt[:], dram["vm"].partition_broadcast(128).squeeze(1))

    # ---- x loads ------------------------------------------------------
    # per-(ktile, tchunk) subtiles so SBUF residency stays tiny and loads
    # pipeline with the projection chunks
    xts = {}
    for tn in ("xk", "xv", "xq"):
        tiles = {}
        for j in range(NJ):
            for k in range(NK):
                xt = xp.tile([128, 512], FR, tag="x", name=f"x_{tn}_{k}_{j}")
                sy.dma_start(
                    xt[:],
                    dram[tn][128 * k : 128 * (k + 1), 512 * j : 512 * (j + 1)],
                )
                tiles[(k, j)] = xt
        xts[tn] = tiles

    # ---- projections + per-head LN ------------------------------------
    def project_ln(pjp, tn, wname, mname, bname, cscale, gname, bnameln, outname):
        # The t-column mask is only materially needed for v (value_mask):
        # masked-KEY kn columns are killed by the -BIG exp bias, and
        # masked-QUERY avn columns are zeroed at the softmax normalizer.
        use_mask = mname == "vm"
        if use_mask:
            mbt = vm_mbt
        fast = not flags["use_bias"]
        outs = []
        tp = None if fast else tpp.tile([128, T], FR, tag="tp")
        for j in range(NJ):
            js = slice(512 * j, 512 * (j + 1))
            tn_out = nbuf.tile(
                [128, 512], FR, tag=f"{outname}{j}", name=f"{outname}{j}"
            )
            outs.append(tn_out)
            pp = pjp.tile([128, 512], FP, tag="pp")
            for k in range(NK):
                te.matmul(
                    pp[:],
                    wT[wname][:, 128 * k : 128 * (k + 1)],
                    xts[tn][(k, j)][:],
                    start=(k == 0),
                    stop=(k == NK - 1),
                )
            if fast:
                # pp already holds diff = p - mean_head(p) (weights folded)
                if use_mask:
                    diff = scr.tile([128, 512], FR, tag="diff")
                    v.tensor_mul(diff[:], pp[:], mbt[:, js])
                    dsq = sqp.tile([128, 512], FR, tag="sq")
                    v.tensor_mul(dsq[:], diff[:], diff[:])
                else:
                    diff = pp
                    dsq = sqp.tile([128, 512], FR, tag="sq")
                    sc.activation(dsq[:], pp[:], AF.Square)
                pvar = pjp.tile([128, 512], FP, tag="pvar")
                te.matmul(pvar[:], m2t[:], dsq[:], start=True, stop=True)
                sd = scr.tile([128, 512], FP, tag="sd")
                sc.activation(sd[:], pvar[:], AF.Sqrt, bias=epst[:])
                rsq = scr.tile([128, 512], FP, tag="rsq")
                v.reciprocal(rsq[:], sd[:])
                v.scalar_tensor_tensor(
                    tn_out[:], diff[:], cscale, rsq[:], op0=OP.mult, op1=OP.mult
                )
            else:
                if use_mask:
                    v.tensor_mul(tp[:, js], pp[:], mbt[:, js])
                else:
                    v.tensor_copy(tp[:, js], pp[:])
                v.tensor_scalar_add(tp[:, js], tp[:, js], bias_tiles[bname][:])
                pmu = pjp.tile([128, 512], FP, tag="pvar")
                te.matmul(pmu[:], m2t[:], tp[:, js], start=True, stop=True)
                diff = scr.tile([128, 512], FR, tag="diff")
                v.tensor_sub(diff[:], tp[:, js], pmu[:])
                dsq = sqp.tile([128, 512], FR, tag="sq")
                v.tensor_mul(dsq[:], diff[:], diff[:])
                pvar = pjp.tile([128, 512], FP, tag="pvar")
                te.matmul(pvar[:], m2t[:], dsq[:], start=True, stop=True)
                sd = scr.tile([128, 512], FP, tag="sd")
                sc.activation(sd[:], pvar[:], AF.Sqrt, bias=epst[:])
                rsq = scr.tile([128, 512], FP, tag="rsq")
                v.reciprocal(rsq[:], sd[:])
                v.scalar_tensor_tensor(
                    tn_out[:], diff[:], cscale, rsq[:], op0=OP.mult, op1=OP.mult
                )
            if flags["use_affine"]:
                v.tensor_scalar(
                    tn_out[:],
                    tn_out[:],
                    ge_tiles[gname][:],
                    ge_tiles[bnameln][:],
                    op0=OP.mult,
                    op1=OP.add,
                )
        return outs

    with tc.tile_pool(name="pj", bufs=3, space="PSUM") as pjp:
        kn = project_ln(pjp, "xk", "wk", "km", "bk", 1.0, "gek", "bek", "kn")
        vn = project_ln(pjp, "xv", "wv", "vm", "bv", 1.0, "gev", "bev", "vn")

        # ---- v transpose -> vaugT [128, 32*65] ------------------------
        # (before q's LN so the transposes overlap it off the critical path)
        vaug = nbuf.tile([128, NI * HPC * 65], FR, tag="vaug")
        vaug3 = vaug[:].rearrange("p (n c) -> p n c", c=65)
        # whole-tile memset to 1.0; the transposed v blocks overwrite cols
        # 0..63 of every 65-block, leaving col 64 as the denominator ones.
        v.memset(vaug[:].bitcast(FP), 1.0)
        with tc.tile_pool(name="ptrv", bufs=2, space="PSUM") as ptrv:
            for i in range(NI):
                pt = ptrv.tile([128, 128], FR, tag="vtr")
                te.transpose(
                    pt[:], vn[i // 4][:, 128 * (i % 4) : 128 * (i % 4 + 1)], identr[:]
                )
                for h in range(HPC):
                    eng = v if (i + h) % 2 == 0 else sc
                    if eng is v:
                        v.tensor_copy(
                            vaug3[:, HPC * i + h, 0:64], pt[:, 64 * h : 64 * (h + 1)]
                        )
                    else:
                        sc.copy(
                            vaug3[:, HPC * i + h, 0:64], pt[:, 64 * h : 64 * (h + 1)]
                        )
        qn = project_ln(pjp, "xq", "wq", "qm", "bq", 1.0 / SCALE, "geq", "beq", "qn")

    # ---- attention -----------------------------------------------------
    # Processed in two tq-halves (jj); within a half both heads interleave
    # so score matmuls (K=64, partition bases 0/64) can run concurrently on
    # PE row-groups and the ACT exp stream stays saturated.
    avn = [
        nbuf.tile([128, 512], FR, tag=f"avn{j}", name=f"avn{j}") for j in range(NJ)
    ]
    late_avsb = {}
    with (
        tc.tile_pool(name="ps", bufs=2, space="PSUM") as pss,
        tc.tile_pool(name="pav", bufs=4, space="PSUM") as pav,
    ):
        for jj in range(2):
            av_tiles = {
                (h, jp): pav.tile([65, 512], FP, tag="av", name=f"av{jj}_{h}_{jp}")
                for h in range(HPC)
                for jp in range(2)
            }
            for i in range(NI):
                isl = slice(128 * i, 128 * (i + 1))
                for h in range(HPC):
                    hs = slice(64 * h, 64 * (h + 1))
                    ps = pss.tile([128, 1024], FP, tag="ps", name=f"ps{jj}_{i}_{h}")
                    kslc = kn[i // 4][hs, 128 * (i % 4) : 128 * (i % 4 + 1)]
                    for jp in range(2):
                        j = 2 * jj + jp
                        te.matmul(
                            ps[:, 512 * jp : 512 * (jp + 1)],
                            kslc,
                            qn[j][hs, :],
                            start=True,
                            stop=True,
                        )
                    nt = nump.tile([128, 1024], FR, tag="numer")
                    sc.activation(nt[:], ps[:], AF.Exp, bias=mbias[:, i : i + 1])
                    for jp in range(2):
                        te.matmul(
                            av_tiles[(h, jp)][:],
                            vaug3[:, HPC * i + h, :],
                            nt[:, 512 * jp : 512 * (jp + 1)],
                            start=(i == 0),
                            stop=(i == NI - 1),
                        )
            for h in range(HPC):
                hs = slice(64 * h, 64 * (h + 1))
                for jp in range(2):
                    j = 2 * jj + jp
                    js = slice(512 * j, 512 * (j + 1))
                    avt = av_tiles[(h, jp)]
                    # drain PSUM immediately so the next jj's chains can
                    # allocate their banks without waiting
                    avsb = rbp.tile(
                        [65, 512], FP, tag="avsb", name=f"avsb{jj}_{h}_{jp}"
                    )
                    v.tensor_copy(avsb[:], avt[:])
                    if jj == 0:
                        # overlapped with the jj=1 attention pass: DRAM-bounce
                        # broadcast of qm/denominator
                        rsb = rsbp.tile([1, 512], FP, tag="rsb")
                        v.reciprocal(rsb[:], avsb[64:65, :])
                        v.tensor_mul(rsb[:], rsb[:], qmrow[:, js])
                        dscr = dramp.tile([1, 512], FP, tag="dscr")
                        sy.dma_start(dscr[:], rsb[:])
                        rb = rbp.tile([64, 512], FP, tag="rb")
                        sy.dma_start(rb[:], dscr[:].partition_broadcast(64).squeeze(1))
                        v.tensor_mul(avn[j][hs, :], avsb[0:64, :], rb[:])
                    else:
                        late_avsb[(h, jp)] = avsb

    # ---- out_proj partial + store -------------------------------------
    # j=0,1 (ready from the first attention pass) go out immediately; the
    # second pass's softmax normalization runs concurrently using a matmul
    # broadcast (PSUM banks are free now), then j=2,3 follow.
    with tc.tile_pool(name="pout", bufs=4, space="PSUM") as poutp:

        def outproj(j):
            js = slice(512 * j, 512 * (j + 1))
            for m in range(4):
                po = poutp.tile([128, 512], FP, tag="pout", name=f"po{j}_{m}")
                te.matmul(
                    po[:],
                    woT[:, 128 * m : 128 * (m + 1)],
                    avn[j][:],
                    start=True,
                    stop=True,
                )
                ot = outp.tile([128, 512], FP, tag="outsb", name=f"ot{j}_{m}")
                if (j * 4 + m) % 2 == 0:
                    v.tensor_copy(ot[:], po[:])
                else:
                    sc.copy(ot[:], po[:])
                sy.dma_start(dram["out"][128 * m : 128 * (m + 1), js], ot[:])

        outproj(0)
        outproj(1)
        for h in range(HPC):
            hs = slice(64 * h, 64 * (h + 1))
            for jp in range(2):
                j = 2 + jp
                js = slice(512 * j, 512 * (j + 1))
                avsb = late_avsb[(h, jp)]
                rsb = rsbp.tile([1, 512], FP, tag="rsb", name=f"rsb2{h}_{jp}")
                v.reciprocal(rsb[:], avsb[64:65, :])
                rsbr = rsbp.tile([1, 512], FR, tag="rsbr", name=f"rsbr{h}_{jp}")
                v.tensor_mul(rsbr[:], rsb[:], qmrow[:, js])
                rbp_ps = poutp.tile([64, 512], FP, tag="rbp", name=f"rbp{h}_{jp}")
                te.matmul(rbp_ps[:], ones64[:], rsbr[:], start=True, stop=True)
                v.tensor_mul(avn[j][hs, :], avsb[0:64, :], rbp_ps[:])
        outproj(2)
        outproj(3)




@functools.lru_cache(maxsize=4)
def _build(use_bias, use_affine, debug_names, reps=1):
    nc = bass.Bass(
        "TRN2", target_bir_lowering=False, debug=False, num_devices=NCORES
    )
    dram = {}
    for tn in ("xq", "xk", "xv"):
        dram[tn] = nc.dram_tensor(tn, [E, T], FR, kind="ExternalInput").ap()
    for wn in ("wq", "wk", "wv"):
        dram[wn] = nc.dram_tensor(wn, [128, E], FP, kind="ExternalInput").ap()
    dram["wo"] = nc.dram_tensor("wo", [E, E], FP, kind="ExternalInput").ap()
    for mn in ("qm", "km", "vm"):
        dram[mn] = nc.dram_tensor(mn, [1, T], FP, kind="ExternalInput").ap()
    dram["kmt"] = nc.dram_tensor("kmt", [128, NI], FP, kind="ExternalInput").ap()
    dram["ident"] = nc.dram_tensor("ident", [128, 128], FP, kind="ExternalInput").ap()
    dram["m2"] = nc.dram_tensor("m2", [128, 128], FR, kind="ExternalInput").ap()
    dram["identr"] = nc.dram_tensor("identr", [128, 128], FR, kind="ExternalInput").ap()
    dram["ones64"] = nc.dram_tensor("ones64", [1, 64], FR, kind="ExternalInput").ap()
    if use_bias:
        for bn in ("bq", "bk", "bv"):
            dram[bn] = nc.dram_tensor(bn, [128, 1], FP, kind="ExternalInput").ap()
    if use_affine:
        for gn in ("geq", "beq", "gek", "bek", "gev", "bev"):
            dram[gn] = nc.dram_tensor(gn, [128, 1], FP, kind="ExternalInput").ap()
    dram["out"] = nc.dram_tensor("out", [E, T], FP, kind="ExternalOutput").ap()
    dbg = frozenset(debug_names.split(",")) - {""} if debug_names else frozenset()
    for dname in dbg:
        dram["dbg_" + dname] = nc.dram_tensor(
            "dbg_" + dname, [128, T], FP, kind="ExternalOutput"
        ).ap()

    flags = {"use_bias": use_bias, "use_affine": use_affine}
    with tile.TileContext(nc) as tc:
        _emit(nc, tc, dram, flags, dbg, reps=reps)
    _split_multiwaits(nc)
    return nc


@functools.lru_cache(maxsize=1)
def _m2_const():
    m2 = np.zeros((128, 128), np.float32)
    m2[:64, :64] = 1.0 / DH
    m2[64:, 64:] = 1.0 / DH
    return m2


def _prep_core_inputs(c, a):
    b, hp = divmod(c, G)
    rs = 128 * hp
    wo_perm = np.concatenate(
        [a["Wo"][:, rs : rs + 128], np.delete(a["Wo"], np.s_[rs : rs + 128], axis=1)],
        axis=1,
    )
    d = {
        "xq": a["q"][b],
        "xk": a["k"][b],
        "xv": a["v"][b],
        "wq": a["Wq"][rs : rs + 128],
        "wk": a["Wk"][rs : rs + 128],
        "wv": a["Wv"][rs : rs + 128],
        "wo": wo_perm,
        "qm": a["query_mask"][b].astype(np.float32)[None, :],
        "km": a["key_mask"][b].astype(np.float32)[None, :],
        "vm": a["value_mask"][b].astype(np.float32)[None, :],
        "kmt": a["key_mask"][b].astype(np.float32).reshape(NI, 128).T,
        "ident": np.eye(128, dtype=np.float32),
        "identr": np.eye(128, dtype=np.float32),
        "ones64": np.ones((1, 64), np.float32),
        "m2": _m2_const(),
    }
    return d


_last_results = None


def kernel(**inputs):
    global _last_results
    a = {k: np.asarray(val) for k, val in inputs.items()}
    use_bias = bool(any(np.any(a[bn] != 0) for bn in ("bq", "bk", "bv")))
    use_affine = bool(
        any(np.any(a[gn] != 1) for gn in ("ln_gq", "ln_gk", "ln_gv"))
        or any(np.any(a[bn] != 0) for bn in ("ln_bq", "ln_bk", "ln_bv"))
    )
    debug_names = os.environ.get("KDEBUG", "")

    nc = _build(use_bias, use_affine, debug_names)

    in_maps = []
    for c in range(NCORES):
        d = _prep_core_inputs(c, a)
        b, hp = divmod(c, G)
        rs = 128 * hp
        if use_bias:
            d["bq"] = a["bq"][rs : rs + 128][:, None]
            d["bk"] = a["bk"][rs : rs + 128][:, None]
            d["bv"] = a["bv"][rs : rs + 128][:, None]
        if use_affine:
            d["geq"] = np.tile(a["ln_gq"], HPC)[:, None]
            d["beq"] = (np.tile(a["ln_bq"], HPC) / SCALE)[:, None]
            d["gek"] = np.tile(a["ln_gk"], HPC)[:, None]
            d["bek"] = np.tile(a["ln_bk"], HPC)[:, None]
            d["gev"] = np.tile(a["ln_gv"], HPC)[:, None]
            d["bev"] = np.tile(a["ln_bv"], HPC)[:, None]
        d = {
            k: np.ascontiguousarray(val, dtype=np.float32) for k, val in d.items()
        }
        in_maps.append(d)

    res = bass_utils.run_bass_kernel_spmd(
        nc,
        in_maps,
        core_ids=list(range(NCORES)),
        trace=os.environ.get("KTRACE", "0") == "1",
    )
    _last_results = res

    out = np.zeros((B, E, T), np.float32)
    bo = a["bo"].astype(np.float32)
    for b in range(B):
        acc = res.results[G * b]["out"].astype(np.float32).copy()
        for c in range(G * b + 1, G * b + G):
            acc += res.results[c]["out"]
        out[b] = acc + bo[:, None]
    return out

